# revision 28
# baseline (speedup 1.0000x reference)
"""Causal self-attention (B=2, T=2048, C=1024, H=16) on 8 Trainium2 NeuronCores.

Sharding: data-parallel over batch (2) x tensor-parallel over heads (4 per
core) = 8 cores. c_attn column-sharded, c_proj row-sharded; each core emits a
partial [C, T] projection output (bf16) that the host sums per batch.

v3 hybrid precision. QKV projection and S^T = K Q^T run in bf16 (Q/K accuracy
drives early-token error, which max-err grading is most sensitive to). P and
V are fp8e4m3 so PV runs in DoubleRow mode (two 128-deep k-tiles contracted
per matmul, ~1.75x bf16 throughput) -- EXCEPT the first k-tile pair (tokens
0..255), which stays bf16: softmax averaging washes fp8 quantization out
everywhere else, but early tokens average over too few terms (measured: fp8
P/V everywhere = 2.5e-2 rel err; first-pair bf16 = 3.9e-3). exp reads two
PSUM banks per instruction and writes fp8 directly ([128,1024] activations
amortize the activation engine's ~550ns fixed cost; fp8 output stores 2x
faster than bf16). wv is pre-scaled x32 (fp8 V range), folded back via the
1/32 ones row in the normalization broadcast. Causal masking: diagonal pair
groups compute full-width S, a gpsimd memset zeroes the all-invalid block
and DVE multiplies the two 128x128 diagonal triangles by a tri mask.
"""

import numpy as np
import ml_dtypes

BF = ml_dtypes.bfloat16
F8 = ml_dtypes.float8_e4m3

B, T, C, H, DH = 2, 2048, 1024, 16, 64
N_CORES = 8
G = 2            # batch split
HPC = 4          # heads per core
TQ = 512         # query strip width
TK = 128         # key tile width
NSTRIP = T // TQ        # 4 query strips
NKT = T // TK           # 16 key tiles
NCT = C // 128          # 8 contraction tiles for qkv
WS = 1.0                # no V pre-scale (fp8 is scale-invariant)
VSLOT = 256             # V3 per-(pair,head) stride: 2 x 128 (64 V + 1 + pad)
VBSLOT = 66             # VB per-(kt,head) stride: 64 V + 1 + pad

_CACHE = {}


def _ensure_runtime():
    import jax
    jax.devices()


def _build(with_bias: bool):
    import concourse.tile as tile
    from concourse import bacc, mybir

    f32 = mybir.dt.float32
    bf16 = mybir.dt.bfloat16
    fp8 = mybir.dt.float8e4
    Exp = mybir.ActivationFunctionType.Exp
    Ident = mybir.ActivationFunctionType.Identity
    DR = mybir.MatmulPerfMode.DoubleRow

    nc = bacc.Bacc("TRN2", target_bir_lowering=False, debug=False,
                   enable_asserts=False, num_devices=N_CORES)

    xT_d = nc.dram_tensor("xT", [C, 512], bf16, kind="ExternalInput").ap()
    wqk_d = nc.dram_tensor("wqk", [C, 512], bf16, kind="ExternalInput").ap()
    wv_d = nc.dram_tensor("wv", [C, 256], bf16, kind="ExternalInput").ap()
    wp_d = nc.dram_tensor("wp", [256, C], bf16, kind="ExternalInput").ap()
    # fp8 pair-interleaved copies for DoubleRow QKV (strips/tiles past the
    # bf16-accurate region)
    x8_d = nc.dram_tensor("x8", [128, 4 * 2 * T], fp8,
                          kind="ExternalInput").ap()
    wqk8_d = nc.dram_tensor("wqk8", [128, 4096], fp8,
                            kind="ExternalInput").ap()
    wv8_d = nc.dram_tensor("wv8", [128, 2048], fp8,
                           kind="ExternalInput").ap()
    if with_bias:
        bqk_d = nc.dram_tensor("bqk", [128, 4], bf16, kind="ExternalInput").ap()
        bv_d = nc.dram_tensor("bv", [1, 256], bf16, kind="ExternalInput").ap()
    out_d = nc.dram_tensor("outT", [C, T], bf16, kind="ExternalOutput").ap()

    with tile.TileContext(nc) as tc:
        with (
            tc.tile_pool(name="persist", bufs=1) as pp,
            tc.tile_pool(name="pP", bufs=26) as pP,
            tc.tile_pool(name="pP0", bufs=6) as pP0,
            tc.tile_pool(name="rrow", bufs=6) as pRR,
            tc.tile_pool(name="bcsb", bufs=6) as pBC,
            tc.tile_pool(name="ob", bufs=4) as pO,
            tc.tile_pool(name="psum", bufs=1, space="PSUM") as psp,
        ):
            # ---- persistent SBUF tensors -------------------------------
            xT = [pp.tile([128, 512], bf16, tag=f"xT{i}", name=f"xT{i}")
                  for i in range(NCT)]
            wqk = [pp.tile([128, 512], bf16, tag=f"wqk{i}", name=f"wqk{i}")
                   for i in range(NCT)]
            wv = [pp.tile([128, 256], bf16, tag=f"wv{i}", name=f"wv{i}")
                  for i in range(NCT)]
            x8 = [pp.tile([128, 2 * T], fp8, tag=f"x8{j}", name=f"x8{j}")
                  for j in range(4)]
            wqk8 = [pp.tile([128, 1024], fp8, tag=f"wqk8{j}",
                            name=f"wqk8{j}") for j in range(4)]
            wv8 = [pp.tile([128, 512], fp8, tag=f"wv8{j}", name=f"wv8{j}")
                   for j in range(4)]
            wp = [pp.tile([128, C], bf16, tag=f"wp{p}", name=f"wp{p}")
                  for p in range(2)]
            # strip-0 Q and k-tiles 0..3 of K in bf16; full fp8 copies
            QT = [pp.tile([128, TQ], bf16, tag=f"QT{j}", name=f"QT{j}")
                  for j in range(2)]
            KTb = [pp.tile([128, TQ], bf16, tag=f"KTb{j}", name=f"KTb{j}")
                   for j in range(2)]
            QT8 = [pp.tile([128, T], fp8, tag=f"QT8{j}", name=f"QT8{j}")
                   for j in range(2)]
            KT8 = [pp.tile([128, T], fp8, tag=f"KT8{j}", name=f"KT8{j}")
                   for j in range(2)]
            V3 = pp.tile([128, 8 * HPC * VSLOT], fp8, tag="V3", name="V3")
            VB = pp.tile([128, 2 * HPC * VBSLOT], bf16, tag="VB", name="VB")
            yT2 = [pp.tile([128, T], bf16, tag=f"yT{p}", name=f"yT{p}")
                   for p in range(2)]
            tri8 = pp.tile([128, 128], fp8, tag="tri8", name="tri8")
            trib = pp.tile([128, 128], bf16, tag="trib", name="trib")
            if with_bias:
                bqk = pp.tile([128, 4], bf16, tag="bqk", name="bqk")
                bv = pp.tile([1, 256], bf16, tag="bv", name="bv")
                ones1 = pp.tile([1, 128], bf16, tag="ones1", name="ones1")

            # ---- input DMAs + constants --------------------------------
            for i in range(NCT):
                nc.sync.dma_start(wqk[i][:], wqk_d[i * 128:(i + 1) * 128, :])
                nc.sync.dma_start(xT[i][:], xT_d[i * 128:(i + 1) * 128, :])
            for j in range(4):
                nc.sync.dma_start(wqk8[j][:],
                                  wqk8_d[:, j * 1024:(j + 1) * 1024])
            for c in range(4):
                for j in range(4):
                    nc.sync.dma_start(
                        x8[j][:].rearrange("p (i t) -> p i t", i=2)
                        [:, :, c * TQ:(c + 1) * TQ],
                        x8_d[:, j * 2 * T:(j + 1) * 2 * T]
                        .rearrange("p (i t) -> p i t", i=2)
                        [:, :, c * TQ:(c + 1) * TQ])
            for i in range(NCT):
                nc.sync.dma_start(wv[i][:], wv_d[i * 128:(i + 1) * 128, :])
            for j in range(4):
                nc.sync.dma_start(wv8[j][:], wv8_d[:, j * 512:(j + 1) * 512])
            for p in range(2):
                nc.sync.dma_start(wp[p][:], wp_d[p * 128:(p + 1) * 128, :])
            # tri[i, j] = 1 if i <= j else 0 (keep k <= q in S^T layout)
            for t in (tri8, trib):
                nc.gpsimd.memset(t[:], 1.0)
                nc.gpsimd.affine_select(
                    out=t[:], in_=t[:], compare_op=mybir.AluOpType.is_ge,
                    fill=0.0, base=0, pattern=[[1, 128]],
                    channel_multiplier=-1)
            if with_bias:
                nc.sync.dma_start(bqk[:], bqk_d[:, :])
                nc.sync.dma_start(bv[:], bv_d[:, :])
                nc.gpsimd.memset(ones1[:], 1.0)
            # V3: zero pad cols then ones columns; VB ones columns
            nc.gpsimd.memset(V3[:], 0.0)
            for j8 in range(8):
                for h in range(HPC):
                    base = (j8 * HPC + h) * VSLOT
                    for i in range(2):
                        nc.gpsimd.memset(
                            V3[:, base + i * 128 + 64: base + i * 128 + 65],
                            1.0)
            for kt in range(2):
                for h in range(HPC):
                    base = (kt * HPC + h) * VBSLOT
                    nc.gpsimd.memset(VB[:, base + 64: base + 65], 1.0)

            P_store = {}

            def A_steps(qt):
                """qk m-tiles + v k-tiles for strip qt (8 steps)."""
                steps = []
                nbias = 1 if with_bias else 0

                def qk_step(mt):
                    def f():
                        ps = psp.tile([128, TQ], f32, tag="big", bufs=2,
                                      name="psA")
                        if qt == 0:
                            for ci in range(NCT):
                                nc.tensor.matmul(
                                    ps[:],
                                    lhsT=wqk[ci][:, mt * 128:(mt + 1) * 128],
                                    rhs=xT[ci][:],
                                    start=(ci == 0), stop=(ci == NCT - 1))
                            dst = QT[mt] if mt < 2 else KTb[mt - 2]
                            if with_bias:
                                nc.scalar.activation(dst[:], ps[:], Ident,
                                                     bias=bqk[:, mt:mt + 1])
                            else:
                                nc.vector.tensor_copy(dst[:], ps[:])
                            if mt >= 2:   # fp8 copy of K tiles 0..3 too
                                nc.vector.tensor_copy(
                                    KT8[mt - 2][:, 0:TQ], ps[:])
                        else:
                            for j in range(4):
                                nc.tensor.matmul(
                                    ps[:],
                                    lhsT=wqk8[j][:].rearrange(
                                        "p (i m) -> p i m", i=2)
                                    [:, :, mt * 128:(mt + 1) * 128],
                                    rhs=x8[j][:].rearrange(
                                        "p (i t) -> p i t", i=2)
                                    [:, :, qt * TQ:(qt + 1) * TQ],
                                    start=(j == 0), stop=(j == 3),
                                    perf_mode=DR)
                            dst = QT8[mt] if mt < 2 else KT8[mt - 2]
                            if with_bias:
                                nc.scalar.activation(
                                    dst[:, qt * TQ:(qt + 1) * TQ], ps[:],
                                    Ident, bias=bqk[:, mt:mt + 1])
                            else:
                                nc.vector.tensor_copy(
                                    dst[:, qt * TQ:(qt + 1) * TQ], ps[:])
                    return f

                def v_step(kt):
                    def f():
                        psv = psp.tile([128, TQ], f32, tag="big", bufs=2,
                                       name="psVt")
                        if kt < 2:
                            for ci in range(NCT):
                                nc.tensor.matmul(
                                    psv[:, 0:256],
                                    lhsT=xT[ci][:, kt * 128:(kt + 1) * 128],
                                    rhs=wv[ci][:, :],
                                    start=(ci == 0),
                                    stop=(ci == NCT + nbias - 1))
                        else:
                            for j in range(4):
                                nc.tensor.matmul(
                                    psv[:, 0:256],
                                    lhsT=x8[j][:].rearrange(
                                        "p (i t) -> p i t", i=2)
                                    [:, :, kt * 128:(kt + 1) * 128],
                                    rhs=wv8[j][:].rearrange(
                                        "p (i m) -> p i m", i=2),
                                    start=(j == 0),
                                    stop=(j == 3 and not with_bias),
                                    perf_mode=DR)
                        if with_bias:
                            nc.tensor.matmul(
                                psv[:, 0:256], lhsT=ones1[0:1, :],
                                rhs=bv[0:1, :], start=False, stop=True,
                                skip_group_check=True)
                        j8 = kt // 2
                        s3 = psv[:, 0:256].rearrange("q (a b) -> q a b", b=64)
                        dst = V3[:].rearrange("q (s v) -> q s v", v=VSLOT)[
                            :, j8 * HPC:(j8 + 1) * HPC,
                            (kt % 2) * 128:(kt % 2) * 128 + 64]
                        nc.vector.tensor_copy(dst, s3)
                        if kt < 2:
                            dstb = VB[:].rearrange(
                                "q (s v) -> q s v", v=VBSLOT)[
                                :, kt * HPC:(kt + 1) * HPC, 0:64]
                            nc.vector.tensor_copy(dstb, s3)
                    return f

                for mt in range(4):
                    steps.append(qk_step(mt))
                for kt in range(4 * qt, 4 * qt + 4):
                    steps.append(v_step(kt))
                return steps

            def S_steps(h, qt):
                """One step per k-tile PAIR: 2 bf16 S matmuls into a 2-bank
                psum group, one [128, 2, N] exp -> P (fp8; bf16 for pair 0),
                diagonal masks."""
                j, i = h // 2, h % 2
                rows = slice(64 * i, 64 * i + 64)
                npair = 2 * (qt + 1)

                def pair_step(jp):
                    def f():
                        dp = jp - 2 * qt        # >=0: diagonal pair index
                        off = max(0, dp) * 256
                        ps = psp.tile([128, 2 * TQ], f32, tag="S", bufs=2,
                                      name="psS")
                        ps3 = ps[:].rearrange("p (i n) -> p i n", i=2)
                        for m in range(2):
                            kt = 2 * jp + m
                            if qt == 0:
                                nc.tensor.matmul(
                                    ps[:, m * TQ + off:(m + 1) * TQ],
                                    lhsT=KTb[j][rows,
                                                kt * 128:(kt + 1) * 128],
                                    rhs=QT[j][rows, off:TQ],
                                    start=True, stop=True)
                            else:
                                nc.tensor.matmul(
                                    ps[:, m * TQ + off:(m + 1) * TQ],
                                    lhsT=KT8[j][rows,
                                                kt * 128:(kt + 1) * 128],
                                    rhs=QT8[j][rows,
                                               qt * TQ + off:(qt + 1) * TQ],
                                    start=True, stop=True)
                        if jp == 0 and qt == 0:
                            Pt = pP0.tile([128, 2 * TQ], bf16, tag="P0",
                                          bufs=6, name="P0t")
                            tri = trib
                        else:
                            Pt = pP.tile([128, 2 * TQ], fp8, tag="P",
                                         bufs=26, name="Pt")
                            tri = tri8
                        Pt3 = Pt[:].rearrange("p (i n) -> p i n", i=2)
                        nc.scalar.activation(Pt3[:, :, off:TQ],
                                             ps3[:, :, off:TQ], Exp,
                                             scale=0.125)
                        if dp >= 0:
                            # odd member: zero the all-invalid 128 block
                            nc.gpsimd.memset(
                                Pt[:, TQ + off:TQ + off + 128], 0.0)
                            # diagonal triangles (gpsimd: keep DVE free)
                            for m in range(2):
                                c = m * TQ + off + m * 128
                                nc.gpsimd.tensor_mul(
                                    Pt[:, c:c + 128],
                                    Pt[:, c:c + 128], tri[:])
                        P_store[(h, qt, jp)] = Pt3
                    return f

                return [pair_step(jp) for jp in range(npair)]

            def PV_steps(h, qt):
                """Two steps per head: (1) PV matmuls + reciprocal DMA chain,
                (2) normalization broadcast + y write. Splitting keeps the
                bc matmul's DMA wait from stalling matmuls queued behind it."""
                j, i = h // 2, h % 2
                npair = 2 * (qt + 1)
                state = {}

                def f_mm():
                    psy = psp.tile([128, TQ], f32, tag="pv", bufs=2,
                                   name="psy")
                    if qt == 0:
                        # pair 0 in strip 0: bf16 matmuls (P and V accurate
                        # where softmax averages over few terms)
                        P0 = P_store[(h, qt, 0)]
                        for m in range(2):
                            nc.tensor.matmul(
                                psy[0:65, :],
                                lhsT=VB[:, (m * HPC + h) * VBSLOT:
                                        (m * HPC + h) * VBSLOT + 65],
                                rhs=P0[:, m, :],
                                start=(m == 0), stop=False,
                                skip_group_check=True)
                        dr0 = 1
                    else:
                        dr0 = 0
                    for jp in range(dr0, npair):
                        dp = jp - 2 * qt
                        off = max(0, dp) * 256
                        base = (jp * HPC + h) * VSLOT
                        nc.tensor.matmul(
                            psy[:, off:TQ],
                            lhsT=V3[:, base:base + VSLOT]
                            .rearrange("p (i v) -> p i v", i=2),
                            rhs=P_store[(h, qt, jp)][:, :, off:TQ],
                            start=(jp == 0 and qt != 0),
                            stop=(jp == npair - 1),
                            perf_mode=DR, skip_group_check=True)
                    drow = pRR.tile([1, TQ], f32, tag="rr", name="drow")
                    nc.vector.tensor_copy(drow[:], psy[64:65, :])
                    rec4 = pRR.tile([128, 4], f32, tag="r4", name="rec4")
                    nc.sync.dma_start(rec4[:, :], drow[0:1, :])
                    rec4b = pRR.tile([128, 4], bf16, tag="r4b", name="rec4b")
                    with nc.allow_low_precision("softmax recip in bf16"):
                        nc.vector.reciprocal(rec4b[:], rec4[:])
                    rrow = pRR.tile([1, TQ], bf16, tag="rrb", name="rrow")
                    nc.sync.dma_start(rrow[0:1, :], rec4b[:, :])
                    state["psy"] = psy
                    state["rrow"] = rrow

                def f_norm():
                    psy, rrow = state["psy"], state["rrow"]
                    bcs = pBC.tile([64, TQ], bf16, tag="bc", name="bcs")
                    nc.gpsimd.partition_broadcast(bcs[:], rrow[0:1, :],
                                                  channels=64)
                    nc.vector.tensor_mul(
                        yT2[j][64 * i:64 * i + 64, qt * TQ:(qt + 1) * TQ],
                        psy[0:64, :], bcs[:])
                    if h == 3:
                        for jp in range(npair):
                            for hh in range(HPC):
                                P_store.pop((hh, qt, jp), None)
                return [f_mm, f_norm]

            def PJ_steps(qt):
                def co_step(co):
                    def f():
                        pso = psp.tile([128, TQ], f32, tag="big", bufs=2,
                                       name="psO")
                        for p in range(2):
                            nc.tensor.matmul(
                                pso[:],
                                lhsT=wp[p][:, co * 128:(co + 1) * 128],
                                rhs=yT2[p][:, qt * TQ:(qt + 1) * TQ],
                                start=(p == 0), stop=(p == 1))
                        ob = pO.tile([128, TQ], bf16, tag="ob", name="ob")
                        nc.any.tensor_copy(ob[:], pso[:])
                        nc.sync.dma_start(
                            out_d[co * 128:(co + 1) * 128,
                                  qt * TQ:(qt + 1) * TQ], ob[:])
                    return f

                return [co_step(co) for co in range(8)]

            def weave(s_list, others):
                if not s_list:
                    for f in others:
                        f()
                    return
                r = len(others) / len(s_list)
                acc, oi = 0.5, 0
                for f in s_list:
                    f()
                    acc += r
                    while acc >= 1.0 and oi < len(others):
                        others[oi]()
                        oi += 1
                        acc -= 1.0
                while oi < len(others):
                    others[oi]()
                    oi += 1

            # ---- software-pipelined emission order ---------------------
            for f in A_steps(0):
                f()
            weave(S_steps(0, 0) + S_steps(1, 0), A_steps(1))
            weave(S_steps(2, 0) + S_steps(3, 0),
                  PV_steps(0, 0) + PV_steps(1, 0))
            weave(S_steps(0, 1) + S_steps(1, 1),
                  PV_steps(2, 0) + PV_steps(3, 0) + A_steps(2))
            weave(S_steps(2, 1) + S_steps(3, 1),
                  PV_steps(0, 1) + PV_steps(1, 1) + PJ_steps(0))
            weave(S_steps(0, 2) + S_steps(1, 2),
                  PV_steps(2, 1) + PV_steps(3, 1) + A_steps(3))
            weave(S_steps(2, 2) + S_steps(3, 2),
                  PV_steps(0, 2) + PV_steps(1, 2) + PJ_steps(1))
            weave(S_steps(0, 3) + S_steps(1, 3),
                  PV_steps(2, 2) + PV_steps(3, 2))
            weave(S_steps(2, 3) + S_steps(3, 3),
                  PV_steps(0, 3) + PV_steps(1, 3) + PJ_steps(2))
            weave([], PV_steps(2, 3) + PV_steps(3, 3) + PJ_steps(3))

    nc.compile()
    return nc


def _get_nc(with_bias: bool):
    key = ("nc", with_bias)
    if key not in _CACHE:
        _ensure_runtime()
        _CACHE[key] = _build(with_bias)
    return _CACHE[key]


def _shard_inputs(x, w_qkv, b_qkv, w_proj, with_bias):
    """Build the 8 per-core input maps."""
    in_maps = []
    for core in range(N_CORES):
        b, g = core // 4, core % 4
        hs = [g * HPC + i for i in range(HPC)]
        q_cols = [w_qkv[:, h * DH:(h + 1) * DH] for h in hs]
        k_cols = [w_qkv[:, C + h * DH: C + (h + 1) * DH] for h in hs]
        v_cols = [w_qkv[:, 2 * C + h * DH: 2 * C + (h + 1) * DH] for h in hs]
        xT = np.ascontiguousarray(x[b].T)                       # [C, T]
        wqk_f = np.concatenate(q_cols + k_cols, axis=1)         # [C, 512]
        wv_f = np.concatenate(v_cols, axis=1) * WS              # [C, 256]

        def pair8(a, scale=1.0):
            """[C, n] -> [128, 4*2*n] fp8, c-tile pairs interleaved."""
            n = a.shape[1]
            p = np.stack([
                np.stack([a[256 * j:256 * j + 128],
                          a[256 * j + 128:256 * j + 256]], axis=1)
                for j in range(4)])                  # [4, 128, 2, n]
            return (p.transpose(1, 0, 2, 3).reshape(128, 4 * 2 * n)
                    * scale).astype(F8)

        m = {
            "xT": xT[:, 0:512].astype(BF),
            "wqk": wqk_f.astype(BF),
            "wv": wv_f.astype(BF),
            "x8": pair8(xT),
            "wqk8": pair8(wqk_f),
            "wv8": pair8(wv_f),
            "wp": np.concatenate(
                [w_proj[h * DH:(h + 1) * DH, :] for h in hs],
                axis=0).astype(BF),
        }
        if with_bias:
            bq = np.concatenate([b_qkv[h * DH:(h + 1) * DH] for h in hs])
            bk = np.concatenate(
                [b_qkv[C + h * DH: C + (h + 1) * DH] for h in hs])
            bqkm = np.zeros((128, 4), np.float32)
            bqkm[:, 0] = bq[0:128]
            bqkm[:, 1] = bq[128:256]
            bqkm[:, 2] = bk[0:128]
            bqkm[:, 3] = bk[128:256]
            m["bqk"] = bqkm.astype(BF)
            bvs = np.concatenate(
                [b_qkv[2 * C + h * DH: 2 * C + (h + 1) * DH] for h in hs])
            m["bv"] = (bvs * WS)[None, :].astype(BF)
        in_maps.append(m)
    return in_maps


def run_on_device(x, w_qkv, b_qkv, w_proj, b_proj, trace=False,
                  trace_kwargs=None):
    """Returns (output [B,T,C] float32, BassKernelResults)."""
    x = np.asarray(x, np.float32)
    w_qkv = np.asarray(w_qkv, np.float32)
    b_qkv = np.asarray(b_qkv, np.float32)
    w_proj = np.asarray(w_proj, np.float32)
    b_proj = np.asarray(b_proj, np.float32)

    with_bias = bool(np.any(b_qkv))
    nc = _get_nc(with_bias)
    in_maps = _shard_inputs(x, w_qkv, b_qkv, w_proj, with_bias)

    from concourse.bass_utils import run_bass_kernel_spmd
    res = run_bass_kernel_spmd(nc, in_maps, core_ids=list(range(N_CORES)),
                               trace=trace, **(trace_kwargs or {}))

    out = np.zeros((B, T, C), np.float64)
    for core in range(N_CORES):
        b = core // 4
        out[b] += res.results[core]["outT"].T.astype(np.float64)
    out += b_proj.astype(np.float64)[None, None, :]
    return out.astype(np.float32), res


def kernel(x, w_qkv, b_qkv, w_proj, b_proj):
    out, _ = run_on_device(x, w_qkv, b_qkv, w_proj, b_proj)
    return out


# revision 29
# speedup vs baseline: 1.7191x; 1.7191x over previous
"""Causal self-attention (B=2, T=2048, C=1024, H=16) on 8 Trainium2 NeuronCores.

Sharding: data-parallel over batch (2) x tensor-parallel over heads (4 per
core) = 8 cores. c_attn column-sharded, c_proj row-sharded; each core emits a
partial [C, T] projection output (bf16) that the host sums per batch.

v3 hybrid precision. QKV projection and S^T = K Q^T run in bf16 (Q/K accuracy
drives early-token error, which max-err grading is most sensitive to). P and
V are fp8e4m3 so PV runs in DoubleRow mode (two 128-deep k-tiles contracted
per matmul, ~1.75x bf16 throughput) -- EXCEPT the first k-tile pair (tokens
0..255), which stays bf16: softmax averaging washes fp8 quantization out
everywhere else, but early tokens average over too few terms (measured: fp8
P/V everywhere = 2.5e-2 rel err; first-pair bf16 = 3.9e-3). exp reads two
PSUM banks per instruction and writes fp8 directly ([128,1024] activations
amortize the activation engine's ~550ns fixed cost; fp8 output stores 2x
faster than bf16). wv is pre-scaled x32 (fp8 V range), folded back via the
1/32 ones row in the normalization broadcast. Causal masking: diagonal pair
groups compute full-width S, a gpsimd memset zeroes the all-invalid block
and DVE multiplies the two 128x128 diagonal triangles by a tri mask.
"""

import numpy as np
import ml_dtypes

BF = ml_dtypes.bfloat16
F8 = ml_dtypes.float8_e4m3

B, T, C, H, DH = 2, 2048, 1024, 16, 64
N_CORES = 8
G = 2            # batch split
HPC = 4          # heads per core
TQ = 512         # query strip width
TK = 128         # key tile width
NSTRIP = T // TQ        # 4 query strips
NKT = T // TK           # 16 key tiles
NCT = C // 128          # 8 contraction tiles for qkv
WS = 32.0               # V pre-scale for fp8 range
VSLOT = 256             # V3 per-(pair,head) stride: 2 x 128 (64 V + 1 + pad)
VBSLOT = 66             # VB per-(kt,head) stride: 64 V + 1 + pad

_CACHE = {}


def _ensure_runtime():
    import jax
    jax.devices()


def _build(with_bias: bool):
    import concourse.tile as tile
    from concourse import bacc, mybir

    f32 = mybir.dt.float32
    bf16 = mybir.dt.bfloat16
    fp8 = mybir.dt.float8e4
    Exp = mybir.ActivationFunctionType.Exp
    Ident = mybir.ActivationFunctionType.Identity
    DR = mybir.MatmulPerfMode.DoubleRow

    nc = bacc.Bacc("TRN2", target_bir_lowering=False, debug=False,
                   enable_asserts=False, num_devices=N_CORES)

    xT_d = nc.dram_tensor("xT", [C, 512], bf16, kind="ExternalInput").ap()
    wqk_d = nc.dram_tensor("wqk", [C, 512], bf16, kind="ExternalInput").ap()
    wv_d = nc.dram_tensor("wv", [C, 256], bf16, kind="ExternalInput").ap()
    wp_d = nc.dram_tensor("wp", [256, C], bf16, kind="ExternalInput").ap()
    # fp8 pair-interleaved copies for DoubleRow QKV (strips/tiles past the
    # bf16-accurate region)
    x8_d = nc.dram_tensor("x8", [128, 4 * 2 * T], fp8,
                          kind="ExternalInput").ap()
    wqk8_d = nc.dram_tensor("wqk8", [128, 4096], fp8,
                            kind="ExternalInput").ap()
    wv8_d = nc.dram_tensor("wv8", [128, 2048], fp8,
                           kind="ExternalInput").ap()
    if with_bias:
        bqk_d = nc.dram_tensor("bqk", [128, 4], bf16, kind="ExternalInput").ap()
        bv_d = nc.dram_tensor("bv", [1, 256], bf16, kind="ExternalInput").ap()
    out_d = nc.dram_tensor("outT", [C, T], bf16, kind="ExternalOutput").ap()

    with tile.TileContext(nc) as tc:
        with (
            tc.tile_pool(name="persist", bufs=1) as pp,
            tc.tile_pool(name="pP", bufs=26) as pP,
            tc.tile_pool(name="pP0", bufs=6) as pP0,
            tc.tile_pool(name="rrow", bufs=6) as pRR,
            tc.tile_pool(name="bcsb", bufs=6) as pBC,
            tc.tile_pool(name="ob", bufs=4) as pO,
            tc.tile_pool(name="psum", bufs=1, space="PSUM") as psp,
        ):
            # ---- persistent SBUF tensors -------------------------------
            xT = [pp.tile([128, 512], bf16, tag=f"xT{i}", name=f"xT{i}")
                  for i in range(NCT)]
            wqk = [pp.tile([128, 512], bf16, tag=f"wqk{i}", name=f"wqk{i}")
                   for i in range(NCT)]
            wv = [pp.tile([128, 256], bf16, tag=f"wv{i}", name=f"wv{i}")
                  for i in range(NCT)]
            x8 = [pp.tile([128, 2 * T], fp8, tag=f"x8{j}", name=f"x8{j}")
                  for j in range(4)]
            wqk8 = [pp.tile([128, 1024], fp8, tag=f"wqk8{j}",
                            name=f"wqk8{j}") for j in range(4)]
            wv8 = [pp.tile([128, 512], fp8, tag=f"wv8{j}", name=f"wv8{j}")
                   for j in range(4)]
            wp = [pp.tile([128, C], bf16, tag=f"wp{p}", name=f"wp{p}")
                  for p in range(2)]
            # strip-0 Q and k-tiles 0..3 of K in bf16; full fp8 copies
            QT = [pp.tile([128, TQ], bf16, tag=f"QT{j}", name=f"QT{j}")
                  for j in range(2)]
            KTb = [pp.tile([128, TQ], bf16, tag=f"KTb{j}", name=f"KTb{j}")
                   for j in range(2)]
            QT8 = [pp.tile([128, T], fp8, tag=f"QT8{j}", name=f"QT8{j}")
                   for j in range(2)]
            KT8 = [pp.tile([128, T], fp8, tag=f"KT8{j}", name=f"KT8{j}")
                   for j in range(2)]
            V3 = pp.tile([128, 8 * HPC * VSLOT], fp8, tag="V3", name="V3")
            VB = pp.tile([128, 2 * HPC * VBSLOT], bf16, tag="VB", name="VB")
            yT2 = [pp.tile([128, T], bf16, tag=f"yT{p}", name=f"yT{p}")
                   for p in range(2)]
            ones64 = pp.tile([1, 64], bf16, tag="ones64", name="ones64")
            tri8 = pp.tile([128, 128], fp8, tag="tri8", name="tri8")
            trib = pp.tile([128, 128], bf16, tag="trib", name="trib")
            if with_bias:
                bqk = pp.tile([128, 4], bf16, tag="bqk", name="bqk")
                bv = pp.tile([1, 256], bf16, tag="bv", name="bv")
                ones1 = pp.tile([1, 128], bf16, tag="ones1", name="ones1")

            # ---- input DMAs + constants --------------------------------
            for i in range(NCT):
                nc.sync.dma_start(wqk[i][:], wqk_d[i * 128:(i + 1) * 128, :])
                nc.sync.dma_start(xT[i][:], xT_d[i * 128:(i + 1) * 128, :])
            for j in range(4):
                nc.sync.dma_start(wqk8[j][:],
                                  wqk8_d[:, j * 1024:(j + 1) * 1024])
            for c in range(4):
                for j in range(4):
                    nc.sync.dma_start(
                        x8[j][:].rearrange("p (i t) -> p i t", i=2)
                        [:, :, c * TQ:(c + 1) * TQ],
                        x8_d[:, j * 2 * T:(j + 1) * 2 * T]
                        .rearrange("p (i t) -> p i t", i=2)
                        [:, :, c * TQ:(c + 1) * TQ])
            for i in range(NCT):
                nc.sync.dma_start(wv[i][:], wv_d[i * 128:(i + 1) * 128, :])
            for j in range(4):
                nc.sync.dma_start(wv8[j][:], wv8_d[:, j * 512:(j + 1) * 512])
            for p in range(2):
                nc.sync.dma_start(wp[p][:], wp_d[p * 128:(p + 1) * 128, :])
            nc.gpsimd.memset(ones64[:], 1.0 / WS)
            # tri[i, j] = 1 if i <= j else 0 (keep k <= q in S^T layout)
            for t in (tri8, trib):
                nc.gpsimd.memset(t[:], 1.0)
                nc.gpsimd.affine_select(
                    out=t[:], in_=t[:], compare_op=mybir.AluOpType.is_ge,
                    fill=0.0, base=0, pattern=[[1, 128]],
                    channel_multiplier=-1)
            if with_bias:
                nc.sync.dma_start(bqk[:], bqk_d[:, :])
                nc.sync.dma_start(bv[:], bv_d[:, :])
                nc.gpsimd.memset(ones1[:], 1.0)
            # V3: zero pad cols then ones columns; VB ones columns
            nc.gpsimd.memset(V3[:], 0.0)
            for j8 in range(8):
                for h in range(HPC):
                    base = (j8 * HPC + h) * VSLOT
                    for i in range(2):
                        nc.gpsimd.memset(
                            V3[:, base + i * 128 + 64: base + i * 128 + 65],
                            1.0)
            for kt in range(2):
                for h in range(HPC):
                    base = (kt * HPC + h) * VBSLOT
                    nc.gpsimd.memset(VB[:, base + 64: base + 65], 1.0)

            P_store = {}

            def A_steps(qt):
                """qk m-tiles + v k-tiles for strip qt (8 steps)."""
                steps = []
                nbias = 1 if with_bias else 0

                def qk_step(mt):
                    def f():
                        ps = psp.tile([128, TQ], f32, tag="big", bufs=1,
                                      name="psA")
                        if qt == 0:
                            for ci in range(NCT):
                                nc.tensor.matmul(
                                    ps[:],
                                    lhsT=wqk[ci][:, mt * 128:(mt + 1) * 128],
                                    rhs=xT[ci][:],
                                    start=(ci == 0), stop=(ci == NCT - 1))
                            dst = QT[mt] if mt < 2 else KTb[mt - 2]
                            if with_bias:
                                nc.scalar.activation(dst[:], ps[:], Ident,
                                                     bias=bqk[:, mt:mt + 1])
                            else:
                                nc.vector.tensor_copy(dst[:], ps[:])
                            if mt >= 2:   # fp8 copy of K tiles 0..3 too
                                nc.vector.tensor_copy(
                                    KT8[mt - 2][:, 0:TQ], ps[:])
                        else:
                            for j in range(4):
                                nc.tensor.matmul(
                                    ps[:],
                                    lhsT=wqk8[j][:].rearrange(
                                        "p (i m) -> p i m", i=2)
                                    [:, :, mt * 128:(mt + 1) * 128],
                                    rhs=x8[j][:].rearrange(
                                        "p (i t) -> p i t", i=2)
                                    [:, :, qt * TQ:(qt + 1) * TQ],
                                    start=(j == 0), stop=(j == 3),
                                    perf_mode=DR)
                            dst = QT8[mt] if mt < 2 else KT8[mt - 2]
                            if with_bias:
                                nc.scalar.activation(
                                    dst[:, qt * TQ:(qt + 1) * TQ], ps[:],
                                    Ident, bias=bqk[:, mt:mt + 1])
                            else:
                                nc.vector.tensor_copy(
                                    dst[:, qt * TQ:(qt + 1) * TQ], ps[:])
                    return f

                def v_step(kt):
                    def f():
                        psv = psp.tile([128, TQ], f32, tag="big", bufs=1,
                                       name="psVt")
                        if kt < 2:
                            for ci in range(NCT):
                                nc.tensor.matmul(
                                    psv[:, 0:256],
                                    lhsT=xT[ci][:, kt * 128:(kt + 1) * 128],
                                    rhs=wv[ci][:, :],
                                    start=(ci == 0),
                                    stop=(ci == NCT + nbias - 1))
                        else:
                            for j in range(4):
                                nc.tensor.matmul(
                                    psv[:, 0:256],
                                    lhsT=x8[j][:].rearrange(
                                        "p (i t) -> p i t", i=2)
                                    [:, :, kt * 128:(kt + 1) * 128],
                                    rhs=wv8[j][:].rearrange(
                                        "p (i m) -> p i m", i=2),
                                    start=(j == 0),
                                    stop=(j == 3 and not with_bias),
                                    perf_mode=DR)
                        if with_bias:
                            nc.tensor.matmul(
                                psv[:, 0:256], lhsT=ones1[0:1, :],
                                rhs=bv[0:1, :], start=False, stop=True,
                                skip_group_check=True)
                        j8 = kt // 2
                        s3 = psv[:, 0:256].rearrange("q (a b) -> q a b", b=64)
                        dst = V3[:].rearrange("q (s v) -> q s v", v=VSLOT)[
                            :, j8 * HPC:(j8 + 1) * HPC,
                            (kt % 2) * 128:(kt % 2) * 128 + 64]
                        nc.vector.tensor_copy(dst, s3)
                        if kt < 2:
                            dstb = VB[:].rearrange(
                                "q (s v) -> q s v", v=VBSLOT)[
                                :, kt * HPC:(kt + 1) * HPC, 0:64]
                            nc.vector.tensor_copy(dstb, s3)
                    return f

                for mt in range(4):
                    steps.append(qk_step(mt))
                for kt in range(4 * qt, 4 * qt + 4):
                    steps.append(v_step(kt))
                return steps

            def S_steps(h, qt):
                """One step per k-tile PAIR: 2 bf16 S matmuls into a 2-bank
                psum group, one [128, 2, N] exp -> P (fp8; bf16 for pair 0),
                diagonal masks."""
                j, i = h // 2, h % 2
                rows = slice(64 * i, 64 * i + 64)
                npair = 2 * (qt + 1)

                def pair_step(jp):
                    def f():
                        dp = jp - 2 * qt        # >=0: diagonal pair index
                        off = max(0, dp) * 256
                        ps = psp.tile([128, 2 * TQ], f32, tag="S", bufs=2,
                                      name="psS")
                        ps3 = ps[:].rearrange("p (i n) -> p i n", i=2)
                        for m in range(2):
                            kt = 2 * jp + m
                            if qt == 0:
                                nc.tensor.matmul(
                                    ps[:, m * TQ + off:(m + 1) * TQ],
                                    lhsT=KTb[j][rows,
                                                kt * 128:(kt + 1) * 128],
                                    rhs=QT[j][rows, off:TQ],
                                    start=True, stop=True)
                            else:
                                nc.tensor.matmul(
                                    ps[:, m * TQ + off:(m + 1) * TQ],
                                    lhsT=KT8[j][rows,
                                                kt * 128:(kt + 1) * 128],
                                    rhs=QT8[j][rows,
                                               qt * TQ + off:(qt + 1) * TQ],
                                    start=True, stop=True)
                        if jp == 0 and qt == 0:
                            Pt = pP0.tile([128, 2 * TQ], bf16, tag="P0",
                                          bufs=6, name="P0t")
                            tri = trib
                        else:
                            Pt = pP.tile([128, 2 * TQ], fp8, tag="P",
                                         bufs=26, name="Pt")
                            tri = tri8
                        Pt3 = Pt[:].rearrange("p (i n) -> p i n", i=2)
                        nc.scalar.activation(Pt3[:, :, off:TQ],
                                             ps3[:, :, off:TQ], Exp,
                                             scale=0.125)
                        if dp >= 0:
                            # odd member: zero the all-invalid 128 block
                            nc.gpsimd.memset(
                                Pt[:, TQ + off:TQ + off + 128], 0.0)
                            # diagonal triangles
                            for m in range(2):
                                c = m * TQ + off + m * 128
                                nc.vector.tensor_mul(
                                    Pt[:, c:c + 128],
                                    Pt[:, c:c + 128], tri[:])
                        P_store[(h, qt, jp)] = Pt3
                    return f

                return [pair_step(jp) for jp in range(npair)]

            def PV_steps(h, qt):
                """Two steps per head: (1) PV matmuls + reciprocal DMA chain,
                (2) normalization broadcast + y write. Splitting keeps the
                bc matmul's DMA wait from stalling matmuls queued behind it."""
                j, i = h // 2, h % 2
                npair = 2 * (qt + 1)
                state = {}

                def f_mm():
                    psy = psp.tile([128, TQ], f32, tag="pv", bufs=2,
                                   name="psy")
                    if qt == 0:
                        # pair 0 in strip 0: bf16 matmuls (P and V accurate
                        # where softmax averages over few terms)
                        P0 = P_store[(h, qt, 0)]
                        for m in range(2):
                            nc.tensor.matmul(
                                psy[0:65, :],
                                lhsT=VB[:, (m * HPC + h) * VBSLOT:
                                        (m * HPC + h) * VBSLOT + 65],
                                rhs=P0[:, m, :],
                                start=(m == 0), stop=False,
                                skip_group_check=True)
                        dr0 = 1
                    else:
                        dr0 = 0
                    for jp in range(dr0, npair):
                        dp = jp - 2 * qt
                        off = max(0, dp) * 256
                        base = (jp * HPC + h) * VSLOT
                        nc.tensor.matmul(
                            psy[:, off:TQ],
                            lhsT=V3[:, base:base + VSLOT]
                            .rearrange("p (i v) -> p i v", i=2),
                            rhs=P_store[(h, qt, jp)][:, :, off:TQ],
                            start=(jp == 0 and qt != 0),
                            stop=(jp == npair - 1),
                            perf_mode=DR, skip_group_check=True)
                    drow = pRR.tile([1, TQ], f32, tag="rr", name="drow")
                    nc.vector.tensor_copy(drow[:], psy[64:65, :])
                    rec4 = pRR.tile([128, 4], f32, tag="r4", name="rec4")
                    nc.sync.dma_start(rec4[:, :], drow[0:1, :])
                    rec4b = pRR.tile([128, 4], bf16, tag="r4b", name="rec4b")
                    with nc.allow_low_precision("softmax recip in bf16"):
                        nc.vector.reciprocal(rec4b[:], rec4[:])
                    rrow = pRR.tile([1, TQ], bf16, tag="rrb", name="rrow")
                    nc.sync.dma_start(rrow[0:1, :], rec4b[:, :])
                    state["psy"] = psy
                    state["rrow"] = rrow

                def f_norm():
                    psy, rrow = state["psy"], state["rrow"]
                    bc = psp.tile([64, TQ], f32, tag="bc", bufs=1,
                                  name="psbc")
                    nc.tensor.matmul(bc[:], lhsT=ones64[:], rhs=rrow[:],
                                     start=True, stop=True)
                    bcs = pBC.tile([64, TQ], bf16, tag="bc", name="bcs")
                    nc.vector.tensor_copy(bcs[:], bc[:])
                    nc.vector.tensor_mul(
                        yT2[j][64 * i:64 * i + 64, qt * TQ:(qt + 1) * TQ],
                        psy[0:64, :], bcs[:])
                    if h == 3:
                        for jp in range(npair):
                            for hh in range(HPC):
                                P_store.pop((hh, qt, jp), None)
                return [f_mm, f_norm]

            def PJ_steps(qt):
                def co_step(co):
                    def f():
                        pso = psp.tile([128, TQ], f32, tag="big", bufs=1,
                                       name="psO")
                        for p in range(2):
                            nc.tensor.matmul(
                                pso[:],
                                lhsT=wp[p][:, co * 128:(co + 1) * 128],
                                rhs=yT2[p][:, qt * TQ:(qt + 1) * TQ],
                                start=(p == 0), stop=(p == 1))
                        ob = pO.tile([128, TQ], bf16, tag="ob", name="ob")
                        nc.any.tensor_copy(ob[:], pso[:])
                        nc.sync.dma_start(
                            out_d[co * 128:(co + 1) * 128,
                                  qt * TQ:(qt + 1) * TQ], ob[:])
                    return f

                return [co_step(co) for co in range(8)]

            def weave(s_list, others):
                if not s_list:
                    for f in others:
                        f()
                    return
                r = len(others) / len(s_list)
                acc, oi = 0.5, 0
                for f in s_list:
                    f()
                    acc += r
                    while acc >= 1.0 and oi < len(others):
                        others[oi]()
                        oi += 1
                        acc -= 1.0
                while oi < len(others):
                    others[oi]()
                    oi += 1

            # ---- software-pipelined emission order ---------------------
            for f in A_steps(0):
                f()
            weave(S_steps(0, 0) + S_steps(1, 0), A_steps(1))
            weave(S_steps(2, 0) + S_steps(3, 0),
                  PV_steps(0, 0) + PV_steps(1, 0))
            weave(S_steps(0, 1) + S_steps(1, 1),
                  PV_steps(2, 0) + PV_steps(3, 0) + A_steps(2))
            weave(S_steps(2, 1) + S_steps(3, 1),
                  PV_steps(0, 1) + PV_steps(1, 1) + PJ_steps(0))
            weave(S_steps(0, 2) + S_steps(1, 2),
                  PV_steps(2, 1) + PV_steps(3, 1) + A_steps(3))
            weave(S_steps(2, 2) + S_steps(3, 2),
                  PV_steps(0, 2) + PV_steps(1, 2) + PJ_steps(1))
            weave(S_steps(0, 3) + S_steps(1, 3),
                  PV_steps(2, 2) + PV_steps(3, 2))
            weave(S_steps(2, 3) + S_steps(3, 3),
                  PV_steps(0, 3) + PV_steps(1, 3) + PJ_steps(2))
            weave([], PV_steps(2, 3) + PV_steps(3, 3) + PJ_steps(3))

    nc.compile()
    return nc


def _get_nc(with_bias: bool):
    key = ("nc", with_bias)
    if key not in _CACHE:
        _ensure_runtime()
        _CACHE[key] = _build(with_bias)
    return _CACHE[key]


def _shard_inputs(x, w_qkv, b_qkv, w_proj, with_bias):
    """Build the 8 per-core input maps."""
    in_maps = []
    for core in range(N_CORES):
        b, g = core // 4, core % 4
        hs = [g * HPC + i for i in range(HPC)]
        q_cols = [w_qkv[:, h * DH:(h + 1) * DH] for h in hs]
        k_cols = [w_qkv[:, C + h * DH: C + (h + 1) * DH] for h in hs]
        v_cols = [w_qkv[:, 2 * C + h * DH: 2 * C + (h + 1) * DH] for h in hs]
        xT = np.ascontiguousarray(x[b].T)                       # [C, T]
        wqk_f = np.concatenate(q_cols + k_cols, axis=1)         # [C, 512]
        wv_f = np.concatenate(v_cols, axis=1) * WS              # [C, 256]

        def pair8(a, scale=1.0):
            """[C, n] -> [128, 4*2*n] fp8, c-tile pairs interleaved."""
            n = a.shape[1]
            p = np.stack([
                np.stack([a[256 * j:256 * j + 128],
                          a[256 * j + 128:256 * j + 256]], axis=1)
                for j in range(4)])                  # [4, 128, 2, n]
            return (p.transpose(1, 0, 2, 3).reshape(128, 4 * 2 * n)
                    * scale).astype(F8)

        m = {
            "xT": xT[:, 0:512].astype(BF),
            "wqk": wqk_f.astype(BF),
            "wv": wv_f.astype(BF),
            "x8": pair8(xT),
            "wqk8": pair8(wqk_f),
            "wv8": pair8(wv_f),
            "wp": np.concatenate(
                [w_proj[h * DH:(h + 1) * DH, :] for h in hs],
                axis=0).astype(BF),
        }
        if with_bias:
            bq = np.concatenate([b_qkv[h * DH:(h + 1) * DH] for h in hs])
            bk = np.concatenate(
                [b_qkv[C + h * DH: C + (h + 1) * DH] for h in hs])
            bqkm = np.zeros((128, 4), np.float32)
            bqkm[:, 0] = bq[0:128]
            bqkm[:, 1] = bq[128:256]
            bqkm[:, 2] = bk[0:128]
            bqkm[:, 3] = bk[128:256]
            m["bqk"] = bqkm.astype(BF)
            bvs = np.concatenate(
                [b_qkv[2 * C + h * DH: 2 * C + (h + 1) * DH] for h in hs])
            m["bv"] = (bvs * WS)[None, :].astype(BF)
        in_maps.append(m)
    return in_maps


def run_on_device(x, w_qkv, b_qkv, w_proj, b_proj, trace=False,
                  trace_kwargs=None):
    """Returns (output [B,T,C] float32, BassKernelResults)."""
    x = np.asarray(x, np.float32)
    w_qkv = np.asarray(w_qkv, np.float32)
    b_qkv = np.asarray(b_qkv, np.float32)
    w_proj = np.asarray(w_proj, np.float32)
    b_proj = np.asarray(b_proj, np.float32)

    with_bias = bool(np.any(b_qkv))
    nc = _get_nc(with_bias)
    in_maps = _shard_inputs(x, w_qkv, b_qkv, w_proj, with_bias)

    from concourse.bass_utils import run_bass_kernel_spmd
    res = run_bass_kernel_spmd(nc, in_maps, core_ids=list(range(N_CORES)),
                               trace=trace, **(trace_kwargs or {}))

    out = np.zeros((B, T, C), np.float64)
    for core in range(N_CORES):
        b = core // 4
        out[b] += res.results[core]["outT"].T.astype(np.float64)
    out += b_proj.astype(np.float64)[None, None, :]
    return out.astype(np.float32), res


def kernel(x, w_qkv, b_qkv, w_proj, b_proj):
    out, _ = run_on_device(x, w_qkv, b_qkv, w_proj, b_proj)
    return out
